# revision 21
# baseline (speedup 1.0000x reference)
"""Trainium2 Bass kernel for nn_BoxCrossAttention_352187318473.

Math: the reference's attention has a single KV token, so the softmax over
the key axis (length 1) is exactly 1.0 and the output is independent of
x / Wp / Wq / Wk.  The whole module collapses to

    o   = ((mish(y @ W1 + b1) @ W2 + b2)[:, KV:] @ Wv + bv) @ Wo + bo
    out[b, c, w, h] = 9 * o[b, c]          (9 = kernel_size**2 positions)

Sharding: output viewed as [B*C, W*H] = [1024, 4096]; core i produces rows
[i*128, (i+1)*128) = batch i//2, channel half i%2.  Each core runs the tiny
MLP chain for its batch (activations as [128,1] columns, weights as natural
[K, M] lhsT tiles), then broadcasts o across the 4096 spatial positions and
DMAs the [128, 4096] result out as fp16 (host upcasts while unsharding).

v4 schedule (cost-model timeline):
  - W2h travels as fp8 e3m4 of (16*W2h): halves the dominant weight DMA
    (1MB -> 512KB).  The x16 scale keeps the values out of e3m4's subnormal
    range; it is descaled exactly by packing 16*b2h and folding 9/16 (vs 9)
    into the on-device Wv@Wo fold, so no extra device ops and no extra
    rounding beyond the e3m4 weight quantization itself (~1.1e-2 rel on the
    fixed inputs vs the 2e-2 gate).
  - ALL fp16 operands ride in ONE first DMA (pk1: y | W1 | b1-as-bitcast |
    WvT | Wo | 16*b2h | bv | bo | 9.0): every pre-W2h compute stage gates on
    a single DMA sem, so the tile scheduler cannot head-block the in-order
    PE queue with a later-arriving pack (which cost ~2.5us in v3).
  - W2h is 2 DMAs (7+1 k-chunks); the tail chunk's sem gates only 4 tail
    matmuls -> kvt copy -> 4 ps_o matmuls -> one [128,512] broadcast.
  - the output store is ONE DMA with a stride-0 (broadcast) source AP: the
    [128, 512] fp16 bc tile is read 8x to fill [128, 4096] DRAM.  Descriptor
    elem stays 1KB (>=512B, no small-elem penalty); no HWDGE serialization.
Weights travel as fp16 except W2h (e3m4); output stored as fp16; measured
end-to-end error vs the f32 reference: ~1.1e-2 absmax-rel.
"""

import numpy as np
import ml_dtypes

import concourse.bacc as bacc
import concourse.tile as tile
from concourse import mybir
from concourse.bass_utils import run_bass_kernel_spmd

F32 = mybir.dt.float32
F16 = mybir.dt.float16
F8 = mybir.dt.float8e3          # e3m4
E3M4 = np.dtype(ml_dtypes.float8_e3m4)
AF = mybir.ActivationFunctionType
ALU = mybir.AluOpType

B, C, W, H = 4, 256, 64, 64
WH = W * H            # 4096
TAU = 256
KV = 512
N_CORES = 8

W2_SCALE = 16.0       # pow2; packed as e3m4(16*W2h), descaled via consts

# fp16 pack1a layout (columns): the m1 path, lands first
PK1_Y = 0                       # y [2]
PK1_W1 = 2                      # W1 colpack [2*1024]
PK1_B1 = PK1_W1 + 2048          # b1 f32-as-f16 [16]
PK1_W = PK1_B1 + 16
# fp16 pack1b layout (columns): fold weights + consts, lands second
PKB_WV = 0                      # WvT colpack [2*512]
PKB_WO = PKB_WV + 1024          # Wo-slice colpack [2*128]
PKB_B2 = PKB_WO + 256           # 16*b2h cols [4]
PKB_BV = PKB_B2 + 4             # bv cols [2]
PKB_BO = PKB_BV + 2             # row0: bo [128] | 9.0
PKB_W = PKB_BO + 129
# fp8 pack2: (16*W2h) row-chunks [8*512]; split 7+1 chunks across two DMAs
PK2_W = 8 * 512
PK2_SPLIT = 7 * 512

OUT_DT = F16
BC_W = 512            # broadcast tile width; store reads it WH//BC_W times

_nc_cache = None


def _build_nc():
    nc = bacc.Bacc(trn_type="TRN2")

    pk1 = nc.dram_tensor("pk1", [128, PK1_W], F16, kind="ExternalInput")
    pkb = nc.dram_tensor("pkb", [128, PKB_W], F16, kind="ExternalInput")
    pk2 = nc.dram_tensor("pk2", [128, PK2_W], F8, kind="ExternalInput")
    outd = nc.dram_tensor("out", [128, WH], OUT_DT, kind="ExternalOutput")

    with tile.TileContext(nc) as tc:
        with (
            tc.tile_pool(name="wp", bufs=1) as wp,
            tc.tile_pool(name="ap", bufs=1) as ap,
            tc.tile_pool(name="pp", bufs=1, space="PSUM") as pp,
            tc.tile_pool(name="ppf", bufs=4, space="PSUM") as ppf,
        ):
            # pk1a (m1 path) first, pk1b (fold path) second -- both windows
            # close well before the W2h sems so only the W2h tail gates the
            # store.  W2h split 7+1 k-chunks so the tail after the last
            # chunk's DMA sem is just 4 matmuls.  HWDGE grant order (pk1a,
            # pkb, p2a, p2b) keeps the transfers back-to-back and in order.
            # all on SP: serialized issue keeps completion order = issue
            # order in BOTH the tile scheduler's internal sim (parallel DMA
            # lanes, no shared-engine contention) and the timeline model
            # (shared HWDGE + DMA_ENGINES) -- otherwise the scheduler
            # reorders the in-order PE queue around the "earlier" pack.
            p1 = wp.tile([128, PK1_W], F16, tag="p1")
            nc.sync.dma_start(out=p1, in_=pk1[:, :])
            pb = wp.tile([128, PKB_W], F16, tag="pb")
            nc.sync.dma_start(out=pb, in_=pkb[:, :])
            p2a = wp.tile([128, PK2_SPLIT], F8, tag="p2a")
            nc.sync.dma_start(out=p2a, in_=pk2[:, :PK2_SPLIT])
            p2b = wp.tile([128, PK2_W - PK2_SPLIT], F8, tag="p2b")
            nc.sync.dma_start(out=p2b, in_=pk2[:, PK2_SPLIT:])

            # zero carrier for the spatial broadcast (no load dependency)
            zc = ap.tile([128, BC_W], F16, tag="zc")
            nc.gpsimd.memset(zc, 0.0)

            y_sb = p1[:, PK1_Y:PK1_Y + 2]
            b1_sb = p1[:, PK1_B1:PK1_B1 + 16].bitcast(F32)   # [128, 8] f32

            def w1(k):                      # [128,1024] chunk k of W1
                return p1[:, PK1_W1 + k * 1024: PK1_W1 + (k + 1) * 1024]

            def w2(k):                      # k-chunk k of 16*W2h: [128, 512]
                if k < 7:
                    return p2a[:, k * 512:(k + 1) * 512]
                return p2b[:, (k - 7) * 512:(k - 6) * 512]

            def wv(j):                      # WvT chunk j: [128, 512]
                return pb[:, PKB_WV + j * 512:PKB_WV + (j + 1) * 512]

            def wo(k):                      # Wo-slice chunk k: [128, 128]
                return pb[:, PKB_WO + k * 128:PKB_WO + (k + 1) * 128]

            b2_sb = pb[:, PKB_B2:PKB_B2 + 4]      # fp16 16*b2h columns
            bv_sb = pb[:, PKB_BV:PKB_BV + 2]      # fp16 bv columns
            boT = pb[0:1, PKB_BO:PKB_BO + 128]    # [1,128] row: bo
            nine = pb[0:1, PKB_BO + 128:PKB_BO + 129]  # [1,1] = 9.0

            # ---- L1: t1[1024] = y @ W1  (8 m-chunks, 2 k-chunks) ----
            # Issued first so the mish chain (the longest post-pk1a path)
            # starts as soon as the pk1a sem fires.
            ps_t1 = pp.tile([128, 8], F32, tag="ps_t1")
            for m in range(8):
                for k in range(2):
                    nc.tensor.matmul(
                        out=ps_t1[:, m:m + 1],
                        lhsT=w1(k)[:, m * 128:(m + 1) * 128],
                        rhs=y_sb[:, k:k + 1],
                        start=(k == 0),
                        stop=(k == 1),
                        skip_group_check=True,
                    )
            # mish(v) = v * tanh(softplus(v)) = v * n/(n+2), n = e^v*(e^v+2):
            # one Exp table only (no mid-kernel table switches).
            t1b = ap.tile([128, 8], F32, tag="t1b")
            nc.vector.tensor_add(out=t1b, in0=ps_t1, in1=b1_sb)
            ex = ap.tile([128, 8], F32, tag="ex")
            nc.scalar.activation(out=ex, in_=t1b, func=AF.Exp)
            ex2 = ap.tile([128, 8], F32, tag="ex2")
            nc.vector.tensor_scalar(
                out=ex2, in0=ex, scalar1=2.0, scalar2=None, op0=ALU.add,
            )
            nn = ap.tile([128, 8], F32, tag="nn")
            nc.vector.tensor_mul(out=nn, in0=ex, in1=ex2)
            dd = ap.tile([128, 8], F32, tag="dd")
            nc.vector.tensor_scalar(
                out=dd, in0=nn, scalar1=2.0, scalar2=None, op0=ALU.add,
            )
            rr = ap.tile([128, 8], F32, tag="rr")
            nc.vector.reciprocal(out=rr, in_=dd)
            qq = ap.tile([128, 8], F32, tag="qq")
            nc.vector.tensor_mul(out=qq, in0=nn, in1=rr)
            m1 = ap.tile([128, 8], F16, tag="m1")
            nc.vector.tensor_mul(out=m1, in0=t1b, in1=qq)

            # 9*bv on ACT (keeps DVE free for the mish/kvt/broadcast chain)
            bv9 = ap.tile([128, 2], F16, tag="bv9")
            nc.scalar.activation(out=bv9, in_=bv_sb, func=AF.Copy, scale=9.0)

            # ---- device fold: wf = (9/16) * Wv @ Wo  (overlaps the W2h
            # DMAs); the /16 descales the x16-packed W2h through kvt.
            # PSUM->SBUF copies alternate ACT/DVE so the four [128,128]
            # copies run pairwise-parallel (wf3 gates the last ps_o matmul).
            wf = []
            for r in range(4):
                ps_f = ppf.tile([128, 128], F32, tag="ps_f")
                for j in range(2):
                    nc.tensor.matmul(
                        out=ps_f[:, :],
                        lhsT=wv(j)[:, r * 128:(r + 1) * 128],
                        rhs=wo(j)[:, :],
                        start=(j == 0),
                        stop=(j == 1),
                    )
                t = ap.tile([128, 128], F16, tag=f"wf{r}")
                if r == 3:
                    # wf3 gates the last ps_o matmul; its copy runs on DVE
                    # (ready-time is after the whole mish chain, so it can't
                    # head-block the in-order DVE queue) while ACT still
                    # works through wf0..2.
                    nc.vector.tensor_scalar(
                        out=t, in0=ps_f, scalar1=9.0 / W2_SCALE, scalar2=None,
                        op0=ALU.mult,
                    )
                else:
                    nc.scalar.activation(
                        out=t, in_=ps_f, func=AF.Copy, scale=9.0 / W2_SCALE)
                wf.append(t)

            # ---- o*9 in one PSUM group:
            #   (16*b2h)@wf + 9bv@Wo + 9*bo first (ready mid-stream), then
            #   the 4 kvt@wf matmuls so only they trail the last W2h chunk --
            ps_o = pp.tile([128, 1], F32, tag="ps_o")
            for m in range(4):
                nc.tensor.matmul(
                    out=ps_o[:, 0:1], lhsT=wf[m][:, :], rhs=b2_sb[:, m:m + 1],
                    start=(m == 0), stop=False, skip_group_check=True,
                )
            for j in range(2):
                nc.tensor.matmul(
                    out=ps_o[:, 0:1], lhsT=wo(j)[:, :], rhs=bv9[:, j:j + 1],
                    start=False, stop=False, skip_group_check=True,
                )
            nc.tensor.matmul(
                out=ps_o[:, 0:1], lhsT=boT, rhs=nine,
                start=False, stop=False, skip_group_check=True,
            )
            # ---- L2: 16*kv[512] = m1 @ (16*W2h)  (4 m-cols, 8 k-chunks) ----
            # m-outer: groups stay sequential per PSUM column (interleaved
            # column groups on one tile accumulate incorrectly).
            ps_kv = pp.tile([128, 4], F32, tag="ps_kv")
            for m in range(4):
                for k in range(8):
                    nc.tensor.matmul(
                        out=ps_kv[:, m:m + 1],
                        lhsT=w2(k)[:, m * 128:(m + 1) * 128],
                        rhs=m1[:, k:k + 1],
                        start=(k == 0),
                        stop=(k == 7),
                        skip_group_check=True,
                    )
            kvt = ap.tile([128, 4], F16, tag="kvt")
            nc.vector.tensor_copy(out=kvt, in_=ps_kv)

            # final 4 matmuls close the o*9 group
            for m in range(4):
                nc.tensor.matmul(
                    out=ps_o[:, 0:1], lhsT=wf[m][:, :], rhs=kvt[:, m:m + 1],
                    start=False, stop=(m == 3), skip_group_check=True,
                )

            # ---- broadcast along free dim + single store ----
            # bc[p, :] = ps_o[p]; scalar operand read straight from PSUM.
            bc = ap.tile([128, BC_W], OUT_DT, tag="bc")
            nc.vector.tensor_scalar(
                out=bc, in0=zc[:, :],
                scalar1=ps_o[:, 0:1], scalar2=None, op0=ALU.add,
            )
            # one DMA: read bc 8x (stride-0 loop) -> [128, 4096] DRAM
            nc.sync.dma_start(
                out=outd[:, :],
                in_=bc[:, :].unsqueeze(1).broadcast_to([128, WH // BC_W, BC_W]),
            )

    return nc


def _host_in_maps(y, W1, b1, W2, b2, Wv, bv, Wo, bo):
    n = N_CORES

    def colpack(mat, kchunks):
        # [K, M] -> [128, kchunks*M], chunk k in cols k*M..(k+1)*M
        K, M = mat.shape
        assert K == kchunks * 128
        return mat.reshape(kchunks, 128, M).transpose(1, 0, 2).reshape(128, -1)

    W2h = W2[:, KV:]
    pk2 = np.ascontiguousarray(
        colpack(W2h * np.float32(W2_SCALE), 8).astype(E3M4))
    w1p = colpack(W1, 2).astype(np.float16)          # [128, 2048]
    wvp = colpack(np.ascontiguousarray(Wv.T), 2).astype(np.float16)  # [128, 1024]
    b1p = np.ascontiguousarray(b1.reshape(8, 128).T.astype(np.float32))

    in_maps = []
    for core in range(n):
        b_i, half = core // 2, core % 2
        ch = slice(half * 128, (half + 1) * 128)
        pk1 = np.zeros((128, PK1_W), np.float16)
        pk1[:, PK1_Y:PK1_Y + 2] = y[b_i].reshape(2, 128).T.astype(np.float16)
        pk1[:, PK1_W1:PK1_B1] = w1p
        pk1[:, PK1_B1:] = b1p.view(np.float16)
        pkb_ = np.zeros((128, PKB_W), np.float16)
        pkb_[:, PKB_WV:PKB_WO] = wvp
        pkb_[:, PKB_WO:PKB_B2] = colpack(
            np.ascontiguousarray(Wo[:, ch]), 2).astype(np.float16)
        pkb_[:, PKB_B2:PKB_B2 + 4] = (
            b2[KV:] * np.float32(W2_SCALE)).reshape(4, 128).T.astype(np.float16)
        pkb_[:, PKB_BV:PKB_BV + 2] = bv.reshape(2, 128).T.astype(np.float16)
        pkb_[0, PKB_BO:PKB_BO + 128] = bo[ch].astype(np.float16)
        pkb_[0, PKB_BO + 128] = np.float16(9.0)
        in_maps.append({"pk1": pk1, "pkb": pkb_, "pk2": pk2})
    return in_maps


def run(inputs, trace=False, **kw):
    global _nc_cache
    if _nc_cache is None:
        _nc_cache = _build_nc()
        _nc_cache.finalize()
    nc = _nc_cache
    in_maps = _host_in_maps(
        np.asarray(inputs["y"], np.float32),
        np.asarray(inputs["W1"], np.float32), np.asarray(inputs["b1"], np.float32),
        np.asarray(inputs["W2"], np.float32), np.asarray(inputs["b2"], np.float32),
        np.asarray(inputs["Wv"], np.float32), np.asarray(inputs["bv"], np.float32),
        np.asarray(inputs["Wo"], np.float32), np.asarray(inputs["bo"], np.float32),
    )
    res = run_bass_kernel_spmd(nc, in_maps, core_ids=list(range(N_CORES)),
                               trace=trace, **kw)
    flat = np.empty((B * C, WH), np.float32)
    for core in range(N_CORES):
        flat[core * 128:(core + 1) * 128] = res.results[core]["out"].astype(np.float32)
    out = flat.reshape(B, C, W, H)
    return out, res


def kernel(**inputs):
    out, _ = run(inputs, trace=False)
    return out


# revision 23
# speedup vs baseline: 1.0760x; 1.0760x over previous
"""Trainium2 Bass kernel for nn_BoxCrossAttention_352187318473.

Math: the reference's attention has a single KV token, so the softmax over
the key axis (length 1) is exactly 1.0 and the output is independent of
x / Wp / Wq / Wk.  The whole module collapses to

    o   = ((mish(y @ W1 + b1) @ W2 + b2)[:, KV:] @ Wv + bv) @ Wo + bo
    out[b, c, w, h] = 9 * o[b, c]          (9 = kernel_size**2 positions)

Sharding: output viewed as [B*C, W*H] = [1024, 4096]; core i produces rows
[i*128, (i+1)*128) = batch i//2, channel half i%2.  Each core runs the tiny
MLP chain for its batch (activations as [128,1] columns, weights as natural
[K, M] lhsT tiles), then broadcasts o across the 4096 spatial positions and
DMAs the [128, 4096] result out as fp16 (host upcasts while unsharding).

v4 schedule (cost-model timeline):
  - W2h travels as fp8 e3m4 of (16*W2h): halves the dominant weight DMA
    (1MB -> 512KB).  The x16 scale keeps the values out of e3m4's subnormal
    range; it is descaled exactly by packing 16*b2h and folding 9/16 (vs 9)
    into the on-device Wv@Wo fold, so no extra device ops and no extra
    rounding beyond the e3m4 weight quantization itself (~1.1e-2 rel on the
    fixed inputs vs the 2e-2 gate).
  - ALL fp16 operands ride in ONE first DMA (pk1: y | W1 | b1-as-bitcast |
    WvT | Wo | 16*b2h | bv | bo | 9.0): every pre-W2h compute stage gates on
    a single DMA sem, so the tile scheduler cannot head-block the in-order
    PE queue with a later-arriving pack (which cost ~2.5us in v3).
  - W2h is 2 DMAs (7+1 k-chunks); the tail chunk's sem gates only 4 tail
    matmuls -> kvt copy -> 4 ps_o matmuls -> one [128,512] broadcast.
  - the output store is ONE DMA with a stride-0 (broadcast) source AP: the
    [128, 512] fp16 bc tile is read 8x to fill [128, 4096] DRAM.  Descriptor
    elem stays 1KB (>=512B, no small-elem penalty); no HWDGE serialization.
Weights travel as fp16 except W2h (e3m4); output stored as fp16; measured
end-to-end error vs the f32 reference: ~1.1e-2 absmax-rel.
"""

import numpy as np
import ml_dtypes

import concourse.bacc as bacc
import concourse.tile as tile
from concourse import mybir
from concourse.bass_utils import run_bass_kernel_spmd

F32 = mybir.dt.float32
F16 = mybir.dt.float16
F8 = mybir.dt.float8e3          # e3m4
E3M4 = np.dtype(ml_dtypes.float8_e3m4)
AF = mybir.ActivationFunctionType
ALU = mybir.AluOpType

B, C, W, H = 4, 256, 64, 64
WH = W * H            # 4096
TAU = 256
KV = 512
N_CORES = 8

W2_SCALE = 16.0       # pow2; packed as e3m4(16*W2h), descaled via consts

# fp16 pack1a layout (columns): the m1 path, lands first
PK1_Y = 0                       # y [2]
PK1_W1 = 2                      # W1 colpack [2*1024]
PK1_B1 = PK1_W1 + 2048          # b1 f32-as-f16 [16]
PK1_W = PK1_B1 + 16
# fp16 pack1b layout (columns): fold weights + consts, lands second
PKB_WV = 0                      # WvT colpack [2*512]
PKB_WO = PKB_WV + 1024          # Wo-slice colpack [2*128]
PKB_B2 = PKB_WO + 256           # 16*b2h cols [4]
PKB_BV = PKB_B2 + 4             # bv cols [2]
PKB_BO = PKB_BV + 2             # row0: bo [128] | 9.0
PKB_W = PKB_BO + 129
# fp8 pack2: (16*W2h) row-chunks [8*512]; split 7+1 chunks across two DMAs
PK2_W = 8 * 512
PK2_SPLIT = 7 * 512

OUT_DT = F16
# broadcast tile width; store reads it WH//BC_W times.  256 fp16 cols =
# 512B descriptor elem -- the smallest size that avoids the cost model's
# small-descriptor 2x latency penalty (and HW's read-modify-write floor).
BC_W = 256

_nc_cache = None


def _build_nc():
    nc = bacc.Bacc(trn_type="TRN2")

    pk1 = nc.dram_tensor("pk1", [128, PK1_W], F16, kind="ExternalInput")
    pkb = nc.dram_tensor("pkb", [128, PKB_W], F16, kind="ExternalInput")
    pk2 = nc.dram_tensor("pk2", [128, PK2_W], F8, kind="ExternalInput")
    outd = nc.dram_tensor("out", [128, WH], OUT_DT, kind="ExternalOutput")

    with tile.TileContext(nc) as tc:
        with (
            tc.tile_pool(name="wp", bufs=1) as wp,
            tc.tile_pool(name="ap", bufs=1) as ap,
            tc.tile_pool(name="pp", bufs=1, space="PSUM") as pp,
            tc.tile_pool(name="ppf", bufs=4, space="PSUM") as ppf,
        ):
            # pk1a (m1 path) first, pk1b (fold path) second -- both windows
            # close well before the W2h sems so only the W2h tail gates the
            # store.  W2h split 7+1 k-chunks so the tail after the last
            # chunk's DMA sem is just 4 matmuls.  HWDGE grant order (pk1a,
            # pkb, p2a, p2b) keeps the transfers back-to-back and in order.
            # all on SP: serialized issue keeps completion order = issue
            # order in BOTH the tile scheduler's internal sim (parallel DMA
            # lanes, no shared-engine contention) and the timeline model
            # (shared HWDGE + DMA_ENGINES) -- otherwise the scheduler
            # reorders the in-order PE queue around the "earlier" pack.
            p1 = wp.tile([128, PK1_W], F16, tag="p1")
            nc.sync.dma_start(out=p1, in_=pk1[:, :])
            pb = wp.tile([128, PKB_W], F16, tag="pb")
            nc.sync.dma_start(out=pb, in_=pkb[:, :])
            p2a = wp.tile([128, PK2_SPLIT], F8, tag="p2a")
            nc.sync.dma_start(out=p2a, in_=pk2[:, :PK2_SPLIT])
            p2b = wp.tile([128, PK2_W - PK2_SPLIT], F8, tag="p2b")
            nc.sync.dma_start(out=p2b, in_=pk2[:, PK2_SPLIT:])

            # zero carrier for the spatial broadcast (no load dependency)
            zc = ap.tile([128, BC_W], F16, tag="zc")
            nc.gpsimd.memset(zc, 0.0)

            y_sb = p1[:, PK1_Y:PK1_Y + 2]
            b1_sb = p1[:, PK1_B1:PK1_B1 + 16].bitcast(F32)   # [128, 8] f32

            def w1(k):                      # [128,1024] chunk k of W1
                return p1[:, PK1_W1 + k * 1024: PK1_W1 + (k + 1) * 1024]

            def w2(k):                      # k-chunk k of 16*W2h: [128, 512]
                if k < 7:
                    return p2a[:, k * 512:(k + 1) * 512]
                return p2b[:, (k - 7) * 512:(k - 6) * 512]

            def wv(j):                      # WvT chunk j: [128, 512]
                return pb[:, PKB_WV + j * 512:PKB_WV + (j + 1) * 512]

            def wo(k):                      # Wo-slice chunk k: [128, 128]
                return pb[:, PKB_WO + k * 128:PKB_WO + (k + 1) * 128]

            b2_sb = pb[:, PKB_B2:PKB_B2 + 4]      # fp16 16*b2h columns
            bv_sb = pb[:, PKB_BV:PKB_BV + 2]      # fp16 bv columns
            boT = pb[0:1, PKB_BO:PKB_BO + 128]    # [1,128] row: bo
            nine = pb[0:1, PKB_BO + 128:PKB_BO + 129]  # [1,1] = 9.0

            # ---- L1: t1[1024] = y @ W1  (8 m-chunks, 2 k-chunks) ----
            # Issued first so the mish chain (the longest post-pk1a path)
            # starts as soon as the pk1a sem fires.
            ps_t1 = pp.tile([128, 8], F32, tag="ps_t1")
            for m in range(8):
                for k in range(2):
                    nc.tensor.matmul(
                        out=ps_t1[:, m:m + 1],
                        lhsT=w1(k)[:, m * 128:(m + 1) * 128],
                        rhs=y_sb[:, k:k + 1],
                        start=(k == 0),
                        stop=(k == 1),
                        skip_group_check=True,
                    )
            # mish(v) = v * tanh(softplus(v)) = v * n/(n+2), n = e^v*(e^v+2):
            # one Exp table only (no mid-kernel table switches).
            t1b = ap.tile([128, 8], F32, tag="t1b")
            nc.vector.tensor_add(out=t1b, in0=ps_t1, in1=b1_sb)
            ex = ap.tile([128, 8], F32, tag="ex")
            nc.scalar.activation(out=ex, in_=t1b, func=AF.Exp)
            ex2 = ap.tile([128, 8], F32, tag="ex2")
            nc.vector.tensor_scalar(
                out=ex2, in0=ex, scalar1=2.0, scalar2=None, op0=ALU.add,
            )
            nn = ap.tile([128, 8], F32, tag="nn")
            nc.vector.tensor_mul(out=nn, in0=ex, in1=ex2)
            dd = ap.tile([128, 8], F32, tag="dd")
            nc.vector.tensor_scalar(
                out=dd, in0=nn, scalar1=2.0, scalar2=None, op0=ALU.add,
            )
            rr = ap.tile([128, 8], F32, tag="rr")
            nc.vector.reciprocal(out=rr, in_=dd)
            qq = ap.tile([128, 8], F32, tag="qq")
            nc.vector.tensor_mul(out=qq, in0=nn, in1=rr)
            m1 = ap.tile([128, 8], F16, tag="m1")
            nc.vector.tensor_mul(out=m1, in0=t1b, in1=qq)

            # 9*bv on ACT (keeps DVE free for the mish/kvt/broadcast chain)
            bv9 = ap.tile([128, 2], F16, tag="bv9")
            nc.scalar.activation(out=bv9, in_=bv_sb, func=AF.Copy, scale=9.0)

            # ---- device fold: wf = (9/16) * Wv @ Wo  (overlaps the W2h
            # DMAs); the /16 descales the x16-packed W2h through kvt.
            # PSUM->SBUF copies alternate ACT/DVE so the four [128,128]
            # copies run pairwise-parallel (wf3 gates the last ps_o matmul).
            wf = []
            for r in range(4):
                ps_f = ppf.tile([128, 128], F32, tag="ps_f")
                for j in range(2):
                    nc.tensor.matmul(
                        out=ps_f[:, :],
                        lhsT=wv(j)[:, r * 128:(r + 1) * 128],
                        rhs=wo(j)[:, :],
                        start=(j == 0),
                        stop=(j == 1),
                    )
                t = ap.tile([128, 128], F16, tag=f"wf{r}")
                nc.scalar.activation(
                    out=t, in_=ps_f, func=AF.Copy, scale=9.0 / W2_SCALE)
                wf.append(t)

            # ---- o*9 in one PSUM group:
            #   (16*b2h)@wf + 9bv@Wo + 9*bo first (ready mid-stream), then
            #   the 4 kvt@wf matmuls so only they trail the last W2h chunk --
            ps_o = pp.tile([128, 1], F32, tag="ps_o")
            for m in range(4):
                nc.tensor.matmul(
                    out=ps_o[:, 0:1], lhsT=wf[m][:, :], rhs=b2_sb[:, m:m + 1],
                    start=(m == 0), stop=False, skip_group_check=True,
                )
            for j in range(2):
                nc.tensor.matmul(
                    out=ps_o[:, 0:1], lhsT=wo(j)[:, :], rhs=bv9[:, j:j + 1],
                    start=False, stop=False, skip_group_check=True,
                )
            nc.tensor.matmul(
                out=ps_o[:, 0:1], lhsT=boT, rhs=nine,
                start=False, stop=False, skip_group_check=True,
            )
            # ---- L2: 16*kv[512] = m1 @ (16*W2h)  (4 m-cols, 8 k-chunks) ----
            # m-outer: groups stay sequential per PSUM column (interleaved
            # column groups on one tile accumulate incorrectly).
            ps_kv = pp.tile([128, 4], F32, tag="ps_kv")
            for m in range(4):
                for k in range(8):
                    nc.tensor.matmul(
                        out=ps_kv[:, m:m + 1],
                        lhsT=w2(k)[:, m * 128:(m + 1) * 128],
                        rhs=m1[:, k:k + 1],
                        start=(k == 0),
                        stop=(k == 7),
                        skip_group_check=True,
                    )
            kvt = ap.tile([128, 4], F16, tag="kvt")
            nc.vector.tensor_copy(out=kvt, in_=ps_kv)

            # final 4 matmuls close the o*9 group
            for m in range(4):
                nc.tensor.matmul(
                    out=ps_o[:, 0:1], lhsT=wf[m][:, :], rhs=kvt[:, m:m + 1],
                    start=False, stop=(m == 3), skip_group_check=True,
                )

            # ---- broadcast along free dim + single store ----
            # bc[p, :] = ps_o[p]; scalar operand read straight from PSUM.
            bc = ap.tile([128, BC_W], OUT_DT, tag="bc")
            nc.vector.tensor_scalar(
                out=bc, in0=zc[:, :],
                scalar1=ps_o[:, 0:1], scalar2=None, op0=ALU.add,
            )
            # one DMA: read bc 8x (stride-0 loop) -> [128, 4096] DRAM
            nc.sync.dma_start(
                out=outd[:, :],
                in_=bc[:, :].unsqueeze(1).broadcast_to([128, WH // BC_W, BC_W]),
            )

    return nc


def _host_in_maps(y, W1, b1, W2, b2, Wv, bv, Wo, bo):
    n = N_CORES

    def colpack(mat, kchunks):
        # [K, M] -> [128, kchunks*M], chunk k in cols k*M..(k+1)*M
        K, M = mat.shape
        assert K == kchunks * 128
        return mat.reshape(kchunks, 128, M).transpose(1, 0, 2).reshape(128, -1)

    W2h = W2[:, KV:]
    pk2 = np.ascontiguousarray(
        colpack(W2h * np.float32(W2_SCALE), 8).astype(E3M4))
    w1p = colpack(W1, 2).astype(np.float16)          # [128, 2048]
    wvp = colpack(np.ascontiguousarray(Wv.T), 2).astype(np.float16)  # [128, 1024]
    b1p = np.ascontiguousarray(b1.reshape(8, 128).T.astype(np.float32))

    in_maps = []
    for core in range(n):
        b_i, half = core // 2, core % 2
        ch = slice(half * 128, (half + 1) * 128)
        pk1 = np.zeros((128, PK1_W), np.float16)
        pk1[:, PK1_Y:PK1_Y + 2] = y[b_i].reshape(2, 128).T.astype(np.float16)
        pk1[:, PK1_W1:PK1_B1] = w1p
        pk1[:, PK1_B1:] = b1p.view(np.float16)
        pkb_ = np.zeros((128, PKB_W), np.float16)
        pkb_[:, PKB_WV:PKB_WO] = wvp
        pkb_[:, PKB_WO:PKB_B2] = colpack(
            np.ascontiguousarray(Wo[:, ch]), 2).astype(np.float16)
        pkb_[:, PKB_B2:PKB_B2 + 4] = (
            b2[KV:] * np.float32(W2_SCALE)).reshape(4, 128).T.astype(np.float16)
        pkb_[:, PKB_BV:PKB_BV + 2] = bv.reshape(2, 128).T.astype(np.float16)
        pkb_[0, PKB_BO:PKB_BO + 128] = bo[ch].astype(np.float16)
        pkb_[0, PKB_BO + 128] = np.float16(9.0)
        in_maps.append({"pk1": pk1, "pkb": pkb_, "pk2": pk2})
    return in_maps


def run(inputs, trace=False, **kw):
    global _nc_cache
    if _nc_cache is None:
        _nc_cache = _build_nc()
        _nc_cache.finalize()
    nc = _nc_cache
    in_maps = _host_in_maps(
        np.asarray(inputs["y"], np.float32),
        np.asarray(inputs["W1"], np.float32), np.asarray(inputs["b1"], np.float32),
        np.asarray(inputs["W2"], np.float32), np.asarray(inputs["b2"], np.float32),
        np.asarray(inputs["Wv"], np.float32), np.asarray(inputs["bv"], np.float32),
        np.asarray(inputs["Wo"], np.float32), np.asarray(inputs["bo"], np.float32),
    )
    res = run_bass_kernel_spmd(nc, in_maps, core_ids=list(range(N_CORES)),
                               trace=trace, **kw)
    flat = np.empty((B * C, WH), np.float32)
    for core in range(N_CORES):
        flat[core * 128:(core + 1) * 128] = res.results[core]["out"].astype(np.float32)
    out = flat.reshape(B, C, W, H)
    return out, res


def kernel(**inputs):
    out, _ = run(inputs, trace=False)
    return out


# revision 25
# speedup vs baseline: 1.0970x; 1.0196x over previous
"""Trainium2 Bass kernel for nn_BoxCrossAttention_352187318473.

Math: the reference's attention has a single KV token, so the softmax over
the key axis (length 1) is exactly 1.0 and the output is independent of
x / Wp / Wq / Wk.  The whole module collapses to

    o   = ((mish(y @ W1 + b1) @ W2 + b2)[:, KV:] @ Wv + bv) @ Wo + bo
    out[b, c, w, h] = 9 * o[b, c]          (9 = kernel_size**2 positions)

Sharding: output viewed as [B*C, W*H] = [1024, 4096]; core i produces rows
[i*128, (i+1)*128) = batch i//2, channel half i%2.  Each core runs the tiny
MLP chain for its batch (activations as [128,1] columns, weights as natural
[K, M] lhsT tiles), then broadcasts o across the 4096 spatial positions and
DMAs the [128, 4096] result out as fp16 (host upcasts while unsharding).

v4 schedule (cost-model timeline):
  - W2h travels as fp8 e3m4 of (16*W2h): halves the dominant weight DMA
    (1MB -> 512KB).  The x16 scale keeps the values out of e3m4's subnormal
    range; it is descaled exactly by packing 16*b2h and folding 9/16 (vs 9)
    into the on-device Wv@Wo fold, so no extra device ops and no extra
    rounding beyond the e3m4 weight quantization itself (~1.1e-2 rel on the
    fixed inputs vs the 2e-2 gate).
  - ALL fp16 operands ride in ONE first DMA (pk1: y | W1 | b1-as-bitcast |
    WvT | Wo | 16*b2h | bv | bo | 9.0): every pre-W2h compute stage gates on
    a single DMA sem, so the tile scheduler cannot head-block the in-order
    PE queue with a later-arriving pack (which cost ~2.5us in v3).
  - W2h is 2 DMAs (7+1 k-chunks); the tail chunk's sem gates only 4 tail
    matmuls -> kvt copy -> 4 ps_o matmuls -> one [128,512] broadcast.
  - the output store is ONE DMA with a stride-0 (broadcast) source AP: the
    [128, 512] fp16 bc tile is read 8x to fill [128, 4096] DRAM.  Descriptor
    elem stays 1KB (>=512B, no small-elem penalty); no HWDGE serialization.
Weights travel as fp16 except W2h (e3m4); output stored as fp16; measured
end-to-end error vs the f32 reference: ~1.1e-2 absmax-rel.
"""

import numpy as np
import ml_dtypes

import concourse.bacc as bacc
import concourse.tile as tile
from concourse import mybir
from concourse.bass_utils import run_bass_kernel_spmd
from concourse.vector_clock import ScopedClock


class _LeanTileContext(tile.TileContext):
    """TileContext whose teardown skips the second all-engine barrier.

    The stock ``_drain_and_barrier`` emits drain -> barrier -> sem clear ->
    barrier.  The final barrier only makes the other engines wait for
    Pool's (cheap, sequencer-only) semaphore range-clear before the program
    ends; nothing afterwards consumes semaphores, so it adds ~230ns of pure
    epilogue to every invocation.  Keep the drain (waits for all DMA
    completion sems), the first barrier, and the sem clear itself.
    """

    def _drain_and_barrier(self, tick_clock, wait_clock):
        drain_inst = self.nc.sync.drain()
        wait_clock.add_sem_waits(
            drain_inst.ins, ScopedClock({None: tick_clock.global_clock})
        )
        self.nc.all_engine_barrier()
        assert self.sems is not None
        popped = self.nc._tile_sem_poison_stack.pop()
        assert popped is self._sem_poison
        self.nc.clear_and_free_semaphores(list(self.sems.allocated().values()))

F32 = mybir.dt.float32
F16 = mybir.dt.float16
F8 = mybir.dt.float8e3          # e3m4
E3M4 = np.dtype(ml_dtypes.float8_e3m4)
AF = mybir.ActivationFunctionType
ALU = mybir.AluOpType

B, C, W, H = 4, 256, 64, 64
WH = W * H            # 4096
TAU = 256
KV = 512
N_CORES = 8

W2_SCALE = 16.0       # pow2; packed as e3m4(16*W2h), descaled via consts

# fp16 pack1a layout (columns): the m1 path, lands first
PK1_Y = 0                       # y [2]
PK1_W1 = 2                      # W1 colpack [2*1024]
PK1_B1 = PK1_W1 + 2048          # b1 f32-as-f16 [16]
PK1_W = PK1_B1 + 16
# fp16 pack1b layout (columns): fold weights + consts, lands second
PKB_WV = 0                      # WvT colpack [2*512]
PKB_WO = PKB_WV + 1024          # Wo-slice colpack [2*128]
PKB_B2 = PKB_WO + 256           # 16*b2h cols [4]
PKB_BV = PKB_B2 + 4             # bv cols [2]
PKB_BO = PKB_BV + 2             # row0: bo [128] | 9.0
PKB_W = PKB_BO + 129
# fp8 pack2: (16*W2h) row-chunks [8*512]; split 7+1 chunks across two DMAs
PK2_W = 8 * 512
PK2_SPLIT = 7 * 512

OUT_DT = F16
# broadcast tile width; store reads it WH//BC_W times.  256 fp16 cols =
# 512B descriptor elem -- the smallest size that avoids the cost model's
# small-descriptor 2x latency penalty (and HW's read-modify-write floor).
BC_W = 256

_nc_cache = None


def _build_nc():
    nc = bacc.Bacc(trn_type="TRN2")

    pk1 = nc.dram_tensor("pk1", [128, PK1_W], F16, kind="ExternalInput")
    pkb = nc.dram_tensor("pkb", [128, PKB_W], F16, kind="ExternalInput")
    pk2 = nc.dram_tensor("pk2", [128, PK2_W], F8, kind="ExternalInput")
    outd = nc.dram_tensor("out", [128, WH], OUT_DT, kind="ExternalOutput")

    with _LeanTileContext(nc) as tc:
        with (
            tc.tile_pool(name="wp", bufs=1) as wp,
            tc.tile_pool(name="ap", bufs=1) as ap,
            tc.tile_pool(name="pp", bufs=1, space="PSUM") as pp,
            tc.tile_pool(name="ppf", bufs=4, space="PSUM") as ppf,
        ):
            # pk1a (m1 path) first, pk1b (fold path) second -- both windows
            # close well before the W2h sems so only the W2h tail gates the
            # store.  W2h split 7+1 k-chunks so the tail after the last
            # chunk's DMA sem is just 4 matmuls.  HWDGE grant order (pk1a,
            # pkb, p2a, p2b) keeps the transfers back-to-back and in order.
            # all on SP: serialized issue keeps completion order = issue
            # order in BOTH the tile scheduler's internal sim (parallel DMA
            # lanes, no shared-engine contention) and the timeline model
            # (shared HWDGE + DMA_ENGINES) -- otherwise the scheduler
            # reorders the in-order PE queue around the "earlier" pack.
            p1 = wp.tile([128, PK1_W], F16, tag="p1")
            nc.sync.dma_start(out=p1, in_=pk1[:, :])
            pb = wp.tile([128, PKB_W], F16, tag="pb")
            nc.sync.dma_start(out=pb, in_=pkb[:, :])
            p2a = wp.tile([128, PK2_SPLIT], F8, tag="p2a")
            nc.sync.dma_start(out=p2a, in_=pk2[:, :PK2_SPLIT])
            p2b = wp.tile([128, PK2_W - PK2_SPLIT], F8, tag="p2b")
            nc.sync.dma_start(out=p2b, in_=pk2[:, PK2_SPLIT:])

            # zero carrier for the spatial broadcast (no load dependency)
            zc = ap.tile([128, BC_W], F16, tag="zc")
            nc.gpsimd.memset(zc, 0.0)

            y_sb = p1[:, PK1_Y:PK1_Y + 2]
            b1_sb = p1[:, PK1_B1:PK1_B1 + 16].bitcast(F32)   # [128, 8] f32

            def w1(k):                      # [128,1024] chunk k of W1
                return p1[:, PK1_W1 + k * 1024: PK1_W1 + (k + 1) * 1024]

            def w2(k):                      # k-chunk k of 16*W2h: [128, 512]
                if k < 7:
                    return p2a[:, k * 512:(k + 1) * 512]
                return p2b[:, (k - 7) * 512:(k - 6) * 512]

            def wv(j):                      # WvT chunk j: [128, 512]
                return pb[:, PKB_WV + j * 512:PKB_WV + (j + 1) * 512]

            def wo(k):                      # Wo-slice chunk k: [128, 128]
                return pb[:, PKB_WO + k * 128:PKB_WO + (k + 1) * 128]

            b2_sb = pb[:, PKB_B2:PKB_B2 + 4]      # fp16 16*b2h columns
            bv_sb = pb[:, PKB_BV:PKB_BV + 2]      # fp16 bv columns
            boT = pb[0:1, PKB_BO:PKB_BO + 128]    # [1,128] row: bo
            nine = pb[0:1, PKB_BO + 128:PKB_BO + 129]  # [1,1] = 9.0

            # ---- L1: t1[1024] = y @ W1  (8 m-chunks, 2 k-chunks) ----
            # Issued first so the mish chain (the longest post-pk1a path)
            # starts as soon as the pk1a sem fires.
            ps_t1 = pp.tile([128, 8], F32, tag="ps_t1")
            for m in range(8):
                for k in range(2):
                    nc.tensor.matmul(
                        out=ps_t1[:, m:m + 1],
                        lhsT=w1(k)[:, m * 128:(m + 1) * 128],
                        rhs=y_sb[:, k:k + 1],
                        start=(k == 0),
                        stop=(k == 1),
                        skip_group_check=True,
                    )
            # mish(v) = v * tanh(softplus(v)) = v * n/(n+2), n = e^v*(e^v+2):
            # one Exp table only (no mid-kernel table switches).
            t1b = ap.tile([128, 8], F32, tag="t1b")
            nc.vector.tensor_add(out=t1b, in0=ps_t1, in1=b1_sb)
            ex = ap.tile([128, 8], F32, tag="ex")
            nc.scalar.activation(out=ex, in_=t1b, func=AF.Exp)
            ex2 = ap.tile([128, 8], F32, tag="ex2")
            nc.vector.tensor_scalar(
                out=ex2, in0=ex, scalar1=2.0, scalar2=None, op0=ALU.add,
            )
            nn = ap.tile([128, 8], F32, tag="nn")
            nc.vector.tensor_mul(out=nn, in0=ex, in1=ex2)
            dd = ap.tile([128, 8], F32, tag="dd")
            nc.vector.tensor_scalar(
                out=dd, in0=nn, scalar1=2.0, scalar2=None, op0=ALU.add,
            )
            rr = ap.tile([128, 8], F32, tag="rr")
            nc.vector.reciprocal(out=rr, in_=dd)
            qq = ap.tile([128, 8], F32, tag="qq")
            nc.vector.tensor_mul(out=qq, in0=nn, in1=rr)
            m1 = ap.tile([128, 8], F16, tag="m1")
            nc.vector.tensor_mul(out=m1, in0=t1b, in1=qq)

            # 9*bv on ACT (keeps DVE free for the mish/kvt/broadcast chain)
            bv9 = ap.tile([128, 2], F16, tag="bv9")
            nc.scalar.activation(out=bv9, in_=bv_sb, func=AF.Copy, scale=9.0)

            # ---- device fold: wf = (9/16) * Wv @ Wo  (overlaps the W2h
            # DMAs); the /16 descales the x16-packed W2h through kvt.
            # PSUM->SBUF copies alternate ACT/DVE so the four [128,128]
            # copies run pairwise-parallel (wf3 gates the last ps_o matmul).
            wf = []
            for r in range(4):
                ps_f = ppf.tile([128, 128], F32, tag="ps_f")
                for j in range(2):
                    nc.tensor.matmul(
                        out=ps_f[:, :],
                        lhsT=wv(j)[:, r * 128:(r + 1) * 128],
                        rhs=wo(j)[:, :],
                        start=(j == 0),
                        stop=(j == 1),
                    )
                t = ap.tile([128, 128], F16, tag=f"wf{r}")
                nc.scalar.activation(
                    out=t, in_=ps_f, func=AF.Copy, scale=9.0 / W2_SCALE)
                wf.append(t)

            # ---- o*9 in one PSUM group:
            #   (16*b2h)@wf + 9bv@Wo + 9*bo first (ready mid-stream), then
            #   the 4 kvt@wf matmuls so only they trail the last W2h chunk --
            ps_o = pp.tile([128, 1], F32, tag="ps_o")
            for m in range(4):
                nc.tensor.matmul(
                    out=ps_o[:, 0:1], lhsT=wf[m][:, :], rhs=b2_sb[:, m:m + 1],
                    start=(m == 0), stop=False, skip_group_check=True,
                )
            for j in range(2):
                nc.tensor.matmul(
                    out=ps_o[:, 0:1], lhsT=wo(j)[:, :], rhs=bv9[:, j:j + 1],
                    start=False, stop=False, skip_group_check=True,
                )
            nc.tensor.matmul(
                out=ps_o[:, 0:1], lhsT=boT, rhs=nine,
                start=False, stop=False, skip_group_check=True,
            )
            # ---- L2: 16*kv[512] = m1 @ (16*W2h)  (4 m-cols, 8 k-chunks) ----
            # m-outer: groups stay sequential per PSUM column (interleaved
            # column groups on one tile accumulate incorrectly).
            ps_kv = pp.tile([128, 4], F32, tag="ps_kv")
            for m in range(4):
                for k in range(8):
                    nc.tensor.matmul(
                        out=ps_kv[:, m:m + 1],
                        lhsT=w2(k)[:, m * 128:(m + 1) * 128],
                        rhs=m1[:, k:k + 1],
                        start=(k == 0),
                        stop=(k == 7),
                        skip_group_check=True,
                    )
            kvt = ap.tile([128, 4], F16, tag="kvt")
            nc.vector.tensor_copy(out=kvt, in_=ps_kv)

            # final 4 matmuls close the o*9 group
            for m in range(4):
                nc.tensor.matmul(
                    out=ps_o[:, 0:1], lhsT=wf[m][:, :], rhs=kvt[:, m:m + 1],
                    start=False, stop=(m == 3), skip_group_check=True,
                )

            # ---- broadcast along free dim + single store ----
            # bc[p, :] = ps_o[p]; scalar operand read straight from PSUM.
            bc = ap.tile([128, BC_W], OUT_DT, tag="bc")
            nc.vector.tensor_scalar(
                out=bc, in0=zc[:, :],
                scalar1=ps_o[:, 0:1], scalar2=None, op0=ALU.add,
            )
            # one DMA: read bc 8x (stride-0 loop) -> [128, 4096] DRAM
            nc.sync.dma_start(
                out=outd[:, :],
                in_=bc[:, :].unsqueeze(1).broadcast_to([128, WH // BC_W, BC_W]),
            )

    return nc


def _host_in_maps(y, W1, b1, W2, b2, Wv, bv, Wo, bo):
    n = N_CORES

    def colpack(mat, kchunks):
        # [K, M] -> [128, kchunks*M], chunk k in cols k*M..(k+1)*M
        K, M = mat.shape
        assert K == kchunks * 128
        return mat.reshape(kchunks, 128, M).transpose(1, 0, 2).reshape(128, -1)

    W2h = W2[:, KV:]
    pk2 = np.ascontiguousarray(
        colpack(W2h * np.float32(W2_SCALE), 8).astype(E3M4))
    w1p = colpack(W1, 2).astype(np.float16)          # [128, 2048]
    wvp = colpack(np.ascontiguousarray(Wv.T), 2).astype(np.float16)  # [128, 1024]
    b1p = np.ascontiguousarray(b1.reshape(8, 128).T.astype(np.float32))

    in_maps = []
    for core in range(n):
        b_i, half = core // 2, core % 2
        ch = slice(half * 128, (half + 1) * 128)
        pk1 = np.zeros((128, PK1_W), np.float16)
        pk1[:, PK1_Y:PK1_Y + 2] = y[b_i].reshape(2, 128).T.astype(np.float16)
        pk1[:, PK1_W1:PK1_B1] = w1p
        pk1[:, PK1_B1:] = b1p.view(np.float16)
        pkb_ = np.zeros((128, PKB_W), np.float16)
        pkb_[:, PKB_WV:PKB_WO] = wvp
        pkb_[:, PKB_WO:PKB_B2] = colpack(
            np.ascontiguousarray(Wo[:, ch]), 2).astype(np.float16)
        pkb_[:, PKB_B2:PKB_B2 + 4] = (
            b2[KV:] * np.float32(W2_SCALE)).reshape(4, 128).T.astype(np.float16)
        pkb_[:, PKB_BV:PKB_BV + 2] = bv.reshape(2, 128).T.astype(np.float16)
        pkb_[0, PKB_BO:PKB_BO + 128] = bo[ch].astype(np.float16)
        pkb_[0, PKB_BO + 128] = np.float16(9.0)
        in_maps.append({"pk1": pk1, "pkb": pkb_, "pk2": pk2})
    return in_maps


def run(inputs, trace=False, **kw):
    global _nc_cache
    if _nc_cache is None:
        _nc_cache = _build_nc()
        _nc_cache.finalize()
    nc = _nc_cache
    in_maps = _host_in_maps(
        np.asarray(inputs["y"], np.float32),
        np.asarray(inputs["W1"], np.float32), np.asarray(inputs["b1"], np.float32),
        np.asarray(inputs["W2"], np.float32), np.asarray(inputs["b2"], np.float32),
        np.asarray(inputs["Wv"], np.float32), np.asarray(inputs["bv"], np.float32),
        np.asarray(inputs["Wo"], np.float32), np.asarray(inputs["bo"], np.float32),
    )
    res = run_bass_kernel_spmd(nc, in_maps, core_ids=list(range(N_CORES)),
                               trace=trace, **kw)
    flat = np.empty((B * C, WH), np.float32)
    for core in range(N_CORES):
        flat[core * 128:(core + 1) * 128] = res.results[core]["out"].astype(np.float32)
    out = flat.reshape(B, C, W, H)
    return out, res


def kernel(**inputs):
    out, _ = run(inputs, trace=False)
    return out


# revision 30
# speedup vs baseline: 1.1224x; 1.0232x over previous
"""Trainium2 Bass kernel for nn_BoxCrossAttention_352187318473.

Math: the reference's attention has a single KV token, so the softmax over
the key axis (length 1) is exactly 1.0 and the output is independent of
x / Wp / Wq / Wk.  The whole module collapses to

    o   = ((mish(y @ W1 + b1) @ W2 + b2)[:, KV:] @ Wv + bv) @ Wo + bo
    out[b, c, w, h] = 9 * o[b, c]          (9 = kernel_size**2 positions)

Sharding: output viewed as [B*C, W*H] = [1024, 4096]; core i produces rows
[i*128, (i+1)*128) = batch i//2, channel half i%2.  Each core runs the tiny
MLP chain for its batch (activations as [128,1] columns, weights as natural
[K, M] lhsT tiles), then broadcasts o across the 4096 spatial positions and
DMAs the [128, 4096] result out as fp16 (host upcasts while unsharding).

v4 schedule (cost-model timeline):
  - W2h travels as fp8 e3m4 of (16*W2h): halves the dominant weight DMA
    (1MB -> 512KB).  The x16 scale keeps the values out of e3m4's subnormal
    range; it is descaled exactly by packing 16*b2h and folding 9/16 (vs 9)
    into the on-device Wv@Wo fold, so no extra device ops and no extra
    rounding beyond the e3m4 weight quantization itself (~1.1e-2 rel on the
    fixed inputs vs the 2e-2 gate).
  - ALL fp16 operands ride in ONE first DMA (pk1: y | W1 | b1-as-bitcast |
    WvT | Wo | 16*b2h | bv | bo | 9.0): every pre-W2h compute stage gates on
    a single DMA sem, so the tile scheduler cannot head-block the in-order
    PE queue with a later-arriving pack (which cost ~2.5us in v3).
  - W2h is 2 DMAs (7+1 k-chunks); the tail chunk's sem gates only 4 tail
    matmuls -> kvt copy -> 4 ps_o matmuls -> one [128,512] broadcast.
  - the output store is ONE DMA with a stride-0 (broadcast) source AP: the
    [128, 512] fp16 bc tile is read 8x to fill [128, 4096] DRAM.  Descriptor
    elem stays 1KB (>=512B, no small-elem penalty); no HWDGE serialization.
Weights travel as fp16 except W2h (e3m4); output stored as fp16; measured
end-to-end error vs the f32 reference: ~1.1e-2 absmax-rel.
"""

import numpy as np
import ml_dtypes

import concourse.bacc as bacc
import concourse.tile as tile
from concourse import mybir
from concourse.bass_utils import run_bass_kernel_spmd
from concourse.vector_clock import ScopedClock


class _LeanBacc(bacc.Bacc):
    """Bacc whose __init__ skips the post-const-memset all-engine barrier.

    Bass.__init__ memsets four const-AP columns on Pool and then barriers
    all engines, which delays the first DMA decode by ~590ns.  This program
    reads no const AP (the Exp bias is an explicit zero column in pk1), so
    the barrier guards nothing here.  The memsets themselves still run.
    """

    _in_init = False

    def __init__(self, *args, **kwargs):
        self._in_init = True
        try:
            super().__init__(*args, **kwargs)
        finally:
            self._in_init = False

    def all_engine_barrier(self, **kwargs):
        if self._in_init:
            return
        return super().all_engine_barrier(**kwargs)


class _LeanTileContext(tile.TileContext):
    """TileContext whose teardown skips the second all-engine barrier.

    The stock ``_drain_and_barrier`` emits drain -> barrier -> sem clear ->
    barrier.  The final barrier only makes the other engines wait for
    Pool's (cheap, sequencer-only) semaphore range-clear before the program
    ends; nothing afterwards consumes semaphores, so it adds ~230ns of pure
    epilogue to every invocation.  Keep the drain (waits for all DMA
    completion sems), the first barrier, and the sem clear itself.
    """

    def _drain_and_barrier(self, tick_clock, wait_clock):
        if not (hasattr(self, "_sem_poison") and self.sems is not None
                and hasattr(self.nc, "_tile_sem_poison_stack")):
            return super()._drain_and_barrier(tick_clock, wait_clock)
        drain_inst = self.nc.sync.drain()
        wait_clock.add_sem_waits(
            drain_inst.ins, ScopedClock({None: tick_clock.global_clock})
        )
        self.nc.all_engine_barrier()
        popped = self.nc._tile_sem_poison_stack.pop()
        assert popped is self._sem_poison
        self.nc.clear_and_free_semaphores(list(self.sems.allocated().values()))

F32 = mybir.dt.float32
F16 = mybir.dt.float16
F8 = mybir.dt.float8e3          # e3m4
E3M4 = np.dtype(ml_dtypes.float8_e3m4)
AF = mybir.ActivationFunctionType
ALU = mybir.AluOpType

B, C, W, H = 4, 256, 64, 64
WH = W * H            # 4096
TAU = 256
KV = 512
N_CORES = 8

W2_SCALE = 16.0       # pow2; packed as e3m4(16*W2h), descaled via consts

# fp16 pack1a layout (columns): the m1 path, lands first
PK1_Y = 0                       # y [2]
PK1_W1 = 2                      # W1 colpack [2*1024]
PK1_B1 = PK1_W1 + 2048          # b1 f32-as-f16 [16]
PK1_Z = PK1_B1 + 16             # zero column (explicit Exp bias) + pad
PK1_W = PK1_Z + 2
# fp16 pack1b layout (columns): fold weights + consts, lands second
PKB_WV = 0                      # WvT colpack [2*512]
PKB_WO = PKB_WV + 1024          # Wo-slice colpack [2*128]
PKB_B2 = PKB_WO + 256           # 16*b2h cols [4]
PKB_BV = PKB_B2 + 4             # bv cols [2]
PKB_BO = PKB_BV + 2             # row0: bo [128] | 9.0
PKB_W = PKB_BO + 129
# fp8 pack2: (16*W2h) row-chunks [8*512]; split 7+1 chunks across two DMAs
PK2_W = 8 * 512
PK2_SPLIT = 7 * 512

OUT_DT = F16
# broadcast tile width; store reads it WH//BC_W times.  256 fp16 cols =
# 512B descriptor elem -- the smallest size that avoids the cost model's
# small-descriptor 2x latency penalty (and HW's read-modify-write floor).
BC_W = 256

_nc_cache = None


def _build_nc():
    nc = _LeanBacc(trn_type="TRN2")

    pk1 = nc.dram_tensor("pk1", [128, PK1_W], F16, kind="ExternalInput")
    pkb = nc.dram_tensor("pkb", [128, PKB_W], F16, kind="ExternalInput")
    pk2 = nc.dram_tensor("pk2", [128, PK2_W], F8, kind="ExternalInput")
    outd = nc.dram_tensor("out", [128, WH], OUT_DT, kind="ExternalOutput")

    with _LeanTileContext(nc) as tc:
        with (
            tc.tile_pool(name="wp", bufs=1) as wp,
            tc.tile_pool(name="ap", bufs=1) as ap,
            tc.tile_pool(name="pp", bufs=1, space="PSUM") as pp,
            tc.tile_pool(name="ppf", bufs=4, space="PSUM") as ppf,
        ):
            # pk1a (m1 path) first, pk1b (fold path) second -- both windows
            # close well before the W2h sems so only the W2h tail gates the
            # store.  W2h split 7+1 k-chunks so the tail after the last
            # chunk's DMA sem is just 4 matmuls.  HWDGE grant order (pk1a,
            # pkb, p2a, p2b) keeps the transfers back-to-back and in order.
            # all on SP: serialized issue keeps completion order = issue
            # order in BOTH the tile scheduler's internal sim (parallel DMA
            # lanes, no shared-engine contention) and the timeline model
            # (shared HWDGE + DMA_ENGINES) -- otherwise the scheduler
            # reorders the in-order PE queue around the "earlier" pack.
            p1 = wp.tile([128, PK1_W], F16, tag="p1")
            nc.sync.dma_start(out=p1, in_=pk1[:, :])
            pb = wp.tile([128, PKB_W], F16, tag="pb")
            nc.sync.dma_start(out=pb, in_=pkb[:, :])
            p2a = wp.tile([128, PK2_SPLIT], F8, tag="p2a")
            nc.sync.dma_start(out=p2a, in_=pk2[:, :PK2_SPLIT])
            p2b = wp.tile([128, PK2_W - PK2_SPLIT], F8, tag="p2b")
            nc.sync.dma_start(out=p2b, in_=pk2[:, PK2_SPLIT:])

            # zero carrier for the spatial broadcast (no load dependency)
            zc = ap.tile([128, BC_W], F16, tag="zc")
            nc.gpsimd.memset(zc, 0.0)

            y_sb = p1[:, PK1_Y:PK1_Y + 2]
            b1_sb = p1[:, PK1_B1:PK1_B1 + 16].bitcast(F32)   # [128, 8] f32

            def w1(k):                      # [128,1024] chunk k of W1
                return p1[:, PK1_W1 + k * 1024: PK1_W1 + (k + 1) * 1024]

            def w2(k):                      # k-chunk k of 16*W2h: [128, 512]
                if k < 7:
                    return p2a[:, k * 512:(k + 1) * 512]
                return p2b[:, (k - 7) * 512:(k - 6) * 512]

            def wv(j):                      # WvT chunk j: [128, 512]
                return pb[:, PKB_WV + j * 512:PKB_WV + (j + 1) * 512]

            def wo(k):                      # Wo-slice chunk k: [128, 128]
                return pb[:, PKB_WO + k * 128:PKB_WO + (k + 1) * 128]

            b2_sb = pb[:, PKB_B2:PKB_B2 + 4]      # fp16 16*b2h columns
            bv_sb = pb[:, PKB_BV:PKB_BV + 2]      # fp16 bv columns
            boT = pb[0:1, PKB_BO:PKB_BO + 128]    # [1,128] row: bo
            nine = pb[0:1, PKB_BO + 128:PKB_BO + 129]  # [1,1] = 9.0

            # ---- L1: t1[1024] = y @ W1  (8 m-chunks, 2 k-chunks) ----
            # Issued first so the mish chain (the longest post-pk1a path)
            # starts as soon as the pk1a sem fires.
            ps_t1 = pp.tile([128, 8], F32, tag="ps_t1")
            for m in range(8):
                for k in range(2):
                    nc.tensor.matmul(
                        out=ps_t1[:, m:m + 1],
                        lhsT=w1(k)[:, m * 128:(m + 1) * 128],
                        rhs=y_sb[:, k:k + 1],
                        start=(k == 0),
                        stop=(k == 1),
                        skip_group_check=True,
                    )
            # mish(v) = v * tanh(softplus(v)) = v * n/(n+2), n = e^v*(e^v+2):
            # one Exp table only (no mid-kernel table switches).
            t1b = ap.tile([128, 8], F32, tag="t1b")
            nc.vector.tensor_add(out=t1b, in0=ps_t1, in1=b1_sb)
            ex = ap.tile([128, 8], F32, tag="ex")
            nc.scalar.activation(out=ex, in_=t1b, func=AF.Exp,
                                 bias=p1[:, PK1_Z:PK1_Z + 1])
            ex2 = ap.tile([128, 8], F32, tag="ex2")
            nc.vector.tensor_scalar(
                out=ex2, in0=ex, scalar1=2.0, scalar2=None, op0=ALU.add,
            )
            nn = ap.tile([128, 8], F32, tag="nn")
            nc.vector.tensor_mul(out=nn, in0=ex, in1=ex2)
            dd = ap.tile([128, 8], F32, tag="dd")
            nc.vector.tensor_scalar(
                out=dd, in0=nn, scalar1=2.0, scalar2=None, op0=ALU.add,
            )
            rr = ap.tile([128, 8], F32, tag="rr")
            nc.vector.reciprocal(out=rr, in_=dd)
            qq = ap.tile([128, 8], F32, tag="qq")
            nc.vector.tensor_mul(out=qq, in0=nn, in1=rr)
            m1 = ap.tile([128, 8], F16, tag="m1")
            nc.vector.tensor_mul(out=m1, in0=t1b, in1=qq)

            # 9*bv on ACT (keeps DVE free for the mish/kvt/broadcast chain)
            bv9 = ap.tile([128, 2], F16, tag="bv9")
            nc.scalar.activation(out=bv9, in_=bv_sb, func=AF.Copy, scale=9.0)

            # ---- device fold: wf = (9/16) * Wv @ Wo  (overlaps the W2h
            # DMAs); the /16 descales the x16-packed W2h through kvt.
            # PSUM->SBUF copies alternate ACT/DVE so the four [128,128]
            # copies run pairwise-parallel (wf3 gates the last ps_o matmul).
            wf = []
            for r in range(4):
                ps_f = ppf.tile([128, 128], F32, tag="ps_f")
                for j in range(2):
                    nc.tensor.matmul(
                        out=ps_f[:, :],
                        lhsT=wv(j)[:, r * 128:(r + 1) * 128],
                        rhs=wo(j)[:, :],
                        start=(j == 0),
                        stop=(j == 1),
                    )
                t = ap.tile([128, 128], F16, tag=f"wf{r}")
                nc.scalar.activation(
                    out=t, in_=ps_f, func=AF.Copy, scale=9.0 / W2_SCALE)
                wf.append(t)

            # ---- o*9 in one PSUM group:
            #   (16*b2h)@wf + 9bv@Wo + 9*bo first (ready mid-stream), then
            #   the 4 kvt@wf matmuls so only they trail the last W2h chunk --
            ps_o = pp.tile([128, 1], F32, tag="ps_o")
            for m in range(4):
                nc.tensor.matmul(
                    out=ps_o[:, 0:1], lhsT=wf[m][:, :], rhs=b2_sb[:, m:m + 1],
                    start=(m == 0), stop=False, skip_group_check=True,
                )
            for j in range(2):
                nc.tensor.matmul(
                    out=ps_o[:, 0:1], lhsT=wo(j)[:, :], rhs=bv9[:, j:j + 1],
                    start=False, stop=False, skip_group_check=True,
                )
            nc.tensor.matmul(
                out=ps_o[:, 0:1], lhsT=boT, rhs=nine,
                start=False, stop=False, skip_group_check=True,
            )
            # ---- L2: 16*kv[512] = m1 @ (16*W2h)  (4 m-cols, 8 k-chunks) ----
            # m-outer: groups stay sequential per PSUM column (interleaved
            # column groups on one tile accumulate incorrectly).
            ps_kv = pp.tile([128, 4], F32, tag="ps_kv")
            for m in range(4):
                for k in range(8):
                    nc.tensor.matmul(
                        out=ps_kv[:, m:m + 1],
                        lhsT=w2(k)[:, m * 128:(m + 1) * 128],
                        rhs=m1[:, k:k + 1],
                        start=(k == 0),
                        stop=(k == 7),
                        skip_group_check=True,
                    )
            kvt = ap.tile([128, 4], F16, tag="kvt")
            nc.vector.tensor_copy(out=kvt, in_=ps_kv)

            # final 4 matmuls close the o*9 group
            for m in range(4):
                nc.tensor.matmul(
                    out=ps_o[:, 0:1], lhsT=wf[m][:, :], rhs=kvt[:, m:m + 1],
                    start=False, stop=(m == 3), skip_group_check=True,
                )

            # ---- broadcast along free dim + single store ----
            # bc[p, :] = ps_o[p]; scalar operand read straight from PSUM.
            bc = ap.tile([128, BC_W], OUT_DT, tag="bc")
            nc.vector.tensor_scalar(
                out=bc, in0=zc[:, :],
                scalar1=ps_o[:, 0:1], scalar2=None, op0=ALU.add,
            )
            # one DMA: read bc 8x (stride-0 loop) -> [128, 4096] DRAM
            nc.sync.dma_start(
                out=outd[:, :],
                in_=bc[:, :].unsqueeze(1).broadcast_to([128, WH // BC_W, BC_W]),
            )

    return nc


def _host_in_maps(y, W1, b1, W2, b2, Wv, bv, Wo, bo):
    n = N_CORES

    def colpack(mat, kchunks):
        # [K, M] -> [128, kchunks*M], chunk k in cols k*M..(k+1)*M
        K, M = mat.shape
        assert K == kchunks * 128
        return mat.reshape(kchunks, 128, M).transpose(1, 0, 2).reshape(128, -1)

    W2h = W2[:, KV:]
    pk2 = np.ascontiguousarray(
        colpack(W2h * np.float32(W2_SCALE), 8).astype(E3M4))
    w1p = colpack(W1, 2).astype(np.float16)          # [128, 2048]
    wvp = colpack(np.ascontiguousarray(Wv.T), 2).astype(np.float16)  # [128, 1024]
    b1p = np.ascontiguousarray(b1.reshape(8, 128).T.astype(np.float32))

    in_maps = []
    for core in range(n):
        b_i, half = core // 2, core % 2
        ch = slice(half * 128, (half + 1) * 128)
        pk1 = np.zeros((128, PK1_W), np.float16)
        pk1[:, PK1_Y:PK1_Y + 2] = y[b_i].reshape(2, 128).T.astype(np.float16)
        pk1[:, PK1_W1:PK1_B1] = w1p
        pk1[:, PK1_B1:PK1_Z] = b1p.view(np.float16)
        pkb_ = np.zeros((128, PKB_W), np.float16)
        pkb_[:, PKB_WV:PKB_WO] = wvp
        pkb_[:, PKB_WO:PKB_B2] = colpack(
            np.ascontiguousarray(Wo[:, ch]), 2).astype(np.float16)
        pkb_[:, PKB_B2:PKB_B2 + 4] = (
            b2[KV:] * np.float32(W2_SCALE)).reshape(4, 128).T.astype(np.float16)
        pkb_[:, PKB_BV:PKB_BV + 2] = bv.reshape(2, 128).T.astype(np.float16)
        pkb_[0, PKB_BO:PKB_BO + 128] = bo[ch].astype(np.float16)
        pkb_[0, PKB_BO + 128] = np.float16(9.0)
        in_maps.append({"pk1": pk1, "pkb": pkb_, "pk2": pk2})
    return in_maps


def run(inputs, trace=False, **kw):
    global _nc_cache
    if _nc_cache is None:
        _nc_cache = _build_nc()
        _nc_cache.finalize()
    nc = _nc_cache
    in_maps = _host_in_maps(
        np.asarray(inputs["y"], np.float32),
        np.asarray(inputs["W1"], np.float32), np.asarray(inputs["b1"], np.float32),
        np.asarray(inputs["W2"], np.float32), np.asarray(inputs["b2"], np.float32),
        np.asarray(inputs["Wv"], np.float32), np.asarray(inputs["bv"], np.float32),
        np.asarray(inputs["Wo"], np.float32), np.asarray(inputs["bo"], np.float32),
    )
    res = run_bass_kernel_spmd(nc, in_maps, core_ids=list(range(N_CORES)),
                               trace=trace, **kw)
    flat = np.empty((B * C, WH), np.float32)
    for core in range(N_CORES):
        flat[core * 128:(core + 1) * 128] = res.results[core]["out"].astype(np.float32)
    out = flat.reshape(B, C, W, H)
    return out, res


def kernel(**inputs):
    out, _ = run(inputs, trace=False)
    return out
